# revision 3
# baseline (speedup 1.0000x reference)
"""Trainium2 Bass kernel for the NICE additive coupling layer.

reference:
    first  = x[:, 0::2]                                # [B, 128]
    second = x[:, 1::2]                                # [B, 128]
    m      = relu(first @ W1 + b1) @ W2 + b2           # [B, 128]
    out[:, 0::2] = first
    out[:, 1::2] = second + m

Sharding: pure data parallel over 8 NeuronCores — each core gets a
contiguous B/8 = 32768-row slice of x; W1/b1/W2/b2 replicated.

Per-core pipeline (per 512-row block, rows on SBUF partitions):
  DMA in  -> deinterleave even cols (cast bf16) -> PE transpose ->
  mm1 (hT = W1c^T @ firstT, bf16) -> relu+b1 (ACT, PSUM->SBUF bf16) ->
  mm2 per 128-row group (m = hT_chunk^T @ W2c, + b2 via rank-1 matmul)
  -> DVE adds m into odd cols of the input tile in place -> DMA out.

The even columns pass through untouched inside the same tile, so DRAM
traffic is the bare minimum: read x once, write out once.
"""

import numpy as np

# ---------------------------------------------------------------------------
# Workaround for this walrus version: its codegen accepts only ONE sync-wait
# command per instruction, but Tile's semaphore assignment attaches several
# (consumers of multiple DMAs, the kernel-tail drain, ...), which codegen
# rejects with "Too many sync wait commands".  Post-pass: hoist all but the
# last wait of every instruction onto standalone EventSemaphore instructions
# inserted immediately before it on the same engine — semantically identical
# (the engine blocks on each wait in order before executing the op).
# ---------------------------------------------------------------------------


def _split_multi_waits(nc):
    import concourse.mybir as mybir

    n_split = 0
    for fn in nc.m.functions:
        for bb in fn.blocks:
            insts = list(bb.instructions)
            out = []
            changed = False
            for ins in insts:
                si = ins.sync_info
                waits = list(si.on_wait) if si is not None else []
                if len(waits) > 1:
                    for k, w in enumerate(waits[:-1]):
                        ev = mybir.InstEventSemaphore(
                            name=f"{ins.name}-evw{k}", engine=ins.engine
                        )
                        ev.sync_info = mybir.SyncInfo(on_wait=[w], on_update=[])
                        ev.debug = ins.debug
                        out.append(ev)
                        n_split += 1
                    si.on_wait = waits[-1:]
                    changed = True
                out.append(ins)
            if changed:
                bb.instructions = out
    return n_split


# Problem shapes (hardcoded per the harness contract).
N_CORES = 8
B, D = 262144, 256
M = D // 2  # 128
H = 256
P = 128  # SBUF partitions
CHUNK = 4  # 128-row sub-blocks per block -> 512 rows/block
ROWS = B // N_CORES  # 32768 rows per core
NBLK = ROWS // (P * CHUNK)  # 64 blocks per core

_NC_CACHE = {}


def build_nc(reps=1):
    """Build the per-core Bass program (identical on all 8 cores).

    reps > 1 repeats the whole kernel body inside one NEFF; used by the
    timing harness to measure steady-state HW time via the slope between
    rep counts.
    """
    if reps in _NC_CACHE:
        return _NC_CACHE[reps]
    import concourse.bass as bass
    import concourse.mybir as mybir
    import concourse.tile as tile
    from concourse.masks import make_identity

    f32 = mybir.dt.float32
    bf16 = mybir.dt.bfloat16
    Relu = mybir.ActivationFunctionType.Relu

    nc = bass.Bass(trn_type="TRN2")
    x = nc.dram_tensor("x", [ROWS, D], f32, kind="ExternalInput")
    w1 = nc.dram_tensor("W1", [M, H], f32, kind="ExternalInput")
    b1 = nc.dram_tensor("b1", [H], f32, kind="ExternalInput")
    w2 = nc.dram_tensor("W2", [H, M], f32, kind="ExternalInput")
    b2 = nc.dram_tensor("b2", [M], f32, kind="ExternalInput")
    out = nc.dram_tensor("out", [ROWS, D], f32, kind="ExternalOutput")

    x_r = x.rearrange("(n p) d -> p n d", p=P)  # [128, 256, 256]
    o_r = out.rearrange("(n p) d -> p n d", p=P)

    with tile.TileContext(nc) as tc:
        with (
            tc.tile_pool(name="consts", bufs=1) as consts,
            tc.tile_pool(name="sbuf", bufs=3) as pool,
            tc.tile_pool(name="psum", bufs=2, space="PSUM") as psum,
            tc.tile_pool(name="psum_m", bufs=4, space="PSUM") as psum_m,
        ):
            # ---- constants, loaded once -------------------------------
            w1f = consts.tile([P, H], f32)
            nc.sync.dma_start(w1f[:], w1[:])
            w1b = consts.tile([P, H], bf16)
            nc.vector.tensor_copy(w1b[:], w1f[:])

            w2f = consts.tile([P, 2, M], f32)
            nc.sync.dma_start(w2f[:], w2.rearrange("(c p) m -> p c m", p=P))
            w2b = consts.tile([P, 2, M], bf16)
            nc.vector.tensor_copy(w2b[:], w2f[:])

            b1s = consts.tile([P, 2], f32)
            nc.sync.dma_start(b1s[:], b1.rearrange("(c p) -> p c", p=P))

            b2f = consts.tile([1, M], f32)
            nc.sync.dma_start(b2f[:1, :], b2[None, :])
            b2b = consts.tile([1, M], bf16)
            nc.vector.tensor_copy(b2b[:], b2f[:])

            ones = consts.tile([1, P], bf16)
            nc.gpsimd.memset(ones[:], 1.0)

            ident = consts.tile([P, P], bf16)
            make_identity(nc, ident[:])

            # ---- main loop -------------------------------------------
            for _ in range(reps):
                for g in range(NBLK):
                    xt = pool.tile([P, CHUNK, D], f32, tag="xt")
                    nc.sync.dma_start(xt[:], x_r[:, g * CHUNK : (g + 1) * CHUNK, :])

                    # even columns, cast to bf16 (Pool engine: 1-input copy)
                    fb = pool.tile([P, CHUNK, M], bf16, tag="fb")
                    nc.gpsimd.tensor_copy(fb[:], xt[:, :, 0:D:2])

                    # PE transpose -> firstT [feat, rows] in PSUM
                    ft = psum.tile([P, CHUNK, M], bf16, tag="ft")
                    for j in range(CHUNK):
                        nc.tensor.transpose(ft[:, j, :], fb[:, j, :], ident[:])
                    fts = pool.tile([P, CHUNK, M], bf16, tag="fts")
                    nc.scalar.copy(fts[:], ft[:])

                    # mm1: hT[c] = W1[:, c]^T @ firstT  -> relu+b1 -> bf16
                    hb = []
                    for c in range(2):
                        hp = psum.tile([P, CHUNK * M], f32, tag="h")
                        nc.tensor.matmul(
                            hp[:], w1b[:, c * P : (c + 1) * P], fts[:, :, :]
                        )
                        hbc = pool.tile([P, CHUNK * M], bf16, tag="hb")
                        nc.scalar.activation(hbc[:], hp[:], Relu, bias=b1s[:, c : c + 1])
                        hb.append(hbc)

                    # mm2 per 128-row group: m = b2 + sum_c hT_c^T @ W2_c
                    for j in range(CHUNK):
                        mp = psum_m.tile([P, M], f32, tag="m")
                        nc.tensor.matmul(mp[:], ones[:], b2b[:], start=True, stop=False)
                        nc.tensor.matmul(
                            mp[:],
                            hb[0][:, j * P : (j + 1) * P],
                            w2b[:, 0, :],
                            start=False,
                            stop=False,
                        )
                        nc.tensor.matmul(
                            mp[:],
                            hb[1][:, j * P : (j + 1) * P],
                            w2b[:, 1, :],
                            start=False,
                            stop=True,
                        )
                        # odd columns += m, in place
                        nc.vector.tensor_add(
                            xt[:, j, 1:D:2], xt[:, j, 1:D:2], mp[:]
                        )

                    nc.sync.dma_start(o_r[:, g * CHUNK : (g + 1) * CHUNK, :], xt[:])

    _split_multi_waits(nc)
    _NC_CACHE[reps] = nc
    return nc


def kernel(x, W1, b1, W2, b2):
    from concourse import bass_utils

    x = np.ascontiguousarray(x, dtype=np.float32)
    W1 = np.ascontiguousarray(W1, dtype=np.float32)
    b1 = np.ascontiguousarray(b1, dtype=np.float32)
    W2 = np.ascontiguousarray(W2, dtype=np.float32)
    b2 = np.ascontiguousarray(b2, dtype=np.float32)

    nc = build_nc(reps=1)
    in_maps = [
        {
            "x": x[i * ROWS : (i + 1) * ROWS],
            "W1": W1,
            "b1": b1,
            "W2": W2,
            "b2": b2,
        }
        for i in range(N_CORES)
    ]
    res = bass_utils.run_bass_kernel_spmd(
        nc, in_maps, core_ids=list(range(N_CORES)), trace=False
    )
    return np.concatenate([res.results[i]["out"] for i in range(N_CORES)], axis=0)


# revision 4
# speedup vs baseline: 29.3164x; 29.3164x over previous
"""Trainium2 Bass kernel for the NICE additive coupling layer.

reference:
    first  = x[:, 0::2]                                # [B, 128]
    second = x[:, 1::2]                                # [B, 128]
    m      = relu(first @ W1 + b1) @ W2 + b2           # [B, 128]
    out[:, 0::2] = first
    out[:, 1::2] = second + m

Sharding: pure data parallel over 8 NeuronCores — each core gets a
contiguous B/8 = 32768-row slice of x; W1/b1/W2/b2 replicated.

Layout: partition p owns the contiguous row span [p*256, (p+1)*256) of
the core's shard, so every DMA moves large contiguous per-partition
spans (16 KB) — small scattered descriptors were measured 25x slower.

Per-core pipeline, per 2048-row super-tile (16 rows/partition):
  one DMA in -> 4x 512-row compute units:
    deinterleave even cols (Pool, cast bf16) -> PE transpose (bf16) ->
    mm1 (hT = W1c^T @ firstT) -> relu+b1 (ACT, PSUM->SBUF bf16) ->
    mm2 per 128-row group (m = hTc^T @ W2c + b2 via rank-1 matmul) ->
    DVE adds m into the odd cols of the input tile in place
  -> one DMA out.

The even columns pass through untouched inside the same tile, so DRAM
traffic is the bare minimum: read x once, write out once.
"""

import numpy as np

# ---------------------------------------------------------------------------
# Workaround for this walrus version: its codegen accepts only ONE sync-wait
# command per instruction, but Tile's semaphore assignment attaches several
# (consumers of multiple DMAs, the kernel-tail drain, ...), which codegen
# rejects with "Too many sync wait commands".  Post-pass: hoist all but the
# last wait of every instruction onto standalone EventSemaphore instructions
# inserted immediately before it on the same engine — semantically identical
# (the engine blocks on each wait in order before executing the op).
# ---------------------------------------------------------------------------


def _split_multi_waits(nc):
    import concourse.mybir as mybir

    n_split = 0
    for fn in nc.m.functions:
        for bb in fn.blocks:
            insts = list(bb.instructions)
            out = []
            changed = False
            for ins in insts:
                si = ins.sync_info
                waits = list(si.on_wait) if si is not None else []
                if len(waits) > 1:
                    for k, w in enumerate(waits[:-1]):
                        ev = mybir.InstEventSemaphore(
                            name=f"{ins.name}-evw{k}", engine=ins.engine
                        )
                        ev.sync_info = mybir.SyncInfo(on_wait=[w], on_update=[])
                        ev.debug = ins.debug
                        out.append(ev)
                        n_split += 1
                    si.on_wait = waits[-1:]
                    changed = True
                out.append(ins)
            if changed:
                bb.instructions = out
    return n_split


# Problem shapes (hardcoded per the harness contract).
N_CORES = 8
B, D = 262144, 256
M = D // 2  # 128
H = 256
P = 128  # SBUF partitions
ROWS = B // N_CORES  # 32768 rows per core
RPP = ROWS // P  # 256 rows owned by each partition
SUP = 16  # rows/partition per super-tile (16 KB DMA spans)
UNIT = 4  # rows/partition per compute unit (512-row matmul blocks)
NSUP = RPP // SUP  # 16 super-tiles per pass
NUNIT = SUP // UNIT  # 4 compute units per super-tile

_NC_CACHE = {}


def build_nc(reps=1):
    """Build the per-core Bass program (identical on all 8 cores).

    reps > 1 wraps the whole pass in a Tile For_i loop; used only by the
    timing harness to measure steady-state HW time via the slope between
    rep counts.
    """
    if reps in _NC_CACHE:
        return _NC_CACHE[reps]
    import concourse.bass as bass
    import concourse.mybir as mybir
    import concourse.tile as tile
    from concourse.masks import make_identity

    f32 = mybir.dt.float32
    bf16 = mybir.dt.bfloat16
    Relu = mybir.ActivationFunctionType.Relu

    nc = bass.Bass(trn_type="TRN2")
    x = nc.dram_tensor("x", [ROWS, D], f32, kind="ExternalInput")
    w1 = nc.dram_tensor("W1", [M, H], f32, kind="ExternalInput")
    b1 = nc.dram_tensor("b1", [H], f32, kind="ExternalInput")
    w2 = nc.dram_tensor("W2", [H, M], f32, kind="ExternalInput")
    b2 = nc.dram_tensor("b2", [M], f32, kind="ExternalInput")
    out = nc.dram_tensor("out", [ROWS, D], f32, kind="ExternalOutput")

    x_r = x.rearrange("(p n) d -> p n d", p=P)  # [128, 256, 256]
    o_r = out.rearrange("(p n) d -> p n d", p=P)

    with tile.TileContext(nc) as tc:
        with (
            tc.tile_pool(name="consts", bufs=1) as consts,
            tc.tile_pool(name="sbuf", bufs=3) as pool,
            tc.tile_pool(name="psum", bufs=2, space="PSUM") as psum,
            tc.tile_pool(name="psum_m", bufs=4, space="PSUM") as psum_m,
        ):
            # ---- constants, loaded once -------------------------------
            w1f = consts.tile([P, H], f32)
            nc.sync.dma_start(w1f[:], w1[:])
            w1b = consts.tile([P, H], bf16)
            nc.vector.tensor_copy(w1b[:], w1f[:])

            w2f = consts.tile([P, 2, M], f32)
            nc.sync.dma_start(w2f[:], w2.rearrange("(c p) m -> p c m", p=P))
            w2b = consts.tile([P, 2, M], bf16)
            nc.vector.tensor_copy(w2b[:], w2f[:])

            b1s = consts.tile([P, 2], f32)
            nc.sync.dma_start(b1s[:], b1.rearrange("(c p) -> p c", p=P))

            b2f = consts.tile([1, M], f32)
            nc.sync.dma_start(b2f[:1, :], b2[None, :])
            b2b = consts.tile([1, M], bf16)
            nc.vector.tensor_copy(b2b[:], b2f[:])

            ones = consts.tile([1, P], bf16)
            nc.gpsimd.memset(ones[:], 1.0)

            ident = consts.tile([P, P], bf16)
            make_identity(nc, ident[:])

            # ---- one full pass over the shard ------------------------
            def one_pass():
                for g in range(NSUP):
                    xt = pool.tile([P, SUP, D], f32, tag="xt")
                    nc.sync.dma_start(xt[:], x_r[:, g * SUP : (g + 1) * SUP, :])

                    for s in range(NUNIT):
                        xu = xt[:, s * UNIT : (s + 1) * UNIT, :]

                        # even columns, cast to bf16 (Pool: 1-input copy)
                        fb = pool.tile([P, UNIT, M], bf16, tag="fb")
                        nc.gpsimd.tensor_copy(fb[:], xu[:, :, 0:D:2])

                        # PE transpose -> firstT [feat, rows] in PSUM
                        ft = psum.tile([P, UNIT, M], bf16, tag="ft")
                        for j in range(UNIT):
                            nc.tensor.transpose(ft[:, j, :], fb[:, j, :], ident[:])
                        fts = pool.tile([P, UNIT, M], bf16, tag="fts")
                        nc.scalar.copy(fts[:], ft[:])

                        # mm1: hT[c] = W1[:, c]^T @ firstT -> relu+b1 -> bf16
                        hb = []
                        for c in range(2):
                            hp = psum.tile([P, UNIT * M], f32, tag="h")
                            nc.tensor.matmul(
                                hp[:], w1b[:, c * P : (c + 1) * P], fts[:, :, :]
                            )
                            hbc = pool.tile([P, UNIT * M], bf16, tag="hb")
                            nc.scalar.activation(
                                hbc[:], hp[:], Relu, bias=b1s[:, c : c + 1]
                            )
                            hb.append(hbc)

                        # mm2 per 128-row group: m = b2 + sum_c hTc^T @ W2c
                        for j in range(UNIT):
                            mp = psum_m.tile([P, M], f32, tag="m")
                            nc.tensor.matmul(
                                mp[:], ones[:], b2b[:], start=True, stop=False
                            )
                            nc.tensor.matmul(
                                mp[:],
                                hb[0][:, j * P : (j + 1) * P],
                                w2b[:, 0, :],
                                start=False,
                                stop=False,
                            )
                            nc.tensor.matmul(
                                mp[:],
                                hb[1][:, j * P : (j + 1) * P],
                                w2b[:, 1, :],
                                start=False,
                                stop=True,
                            )
                            # odd columns += m, in place
                            nc.vector.tensor_add(
                                xu[:, j, 1:D:2], xu[:, j, 1:D:2], mp[:]
                            )

                    nc.sync.dma_start(o_r[:, g * SUP : (g + 1) * SUP, :], xt[:])

            if reps == 1:
                one_pass()
            else:
                with tc.For_i(0, reps, 1):
                    one_pass()

    _split_multi_waits(nc)
    _NC_CACHE[reps] = nc
    return nc


def kernel(x, W1, b1, W2, b2):
    from concourse import bass_utils

    x = np.ascontiguousarray(x, dtype=np.float32)
    W1 = np.ascontiguousarray(W1, dtype=np.float32)
    b1 = np.ascontiguousarray(b1, dtype=np.float32)
    W2 = np.ascontiguousarray(W2, dtype=np.float32)
    b2 = np.ascontiguousarray(b2, dtype=np.float32)

    nc = build_nc(reps=1)
    in_maps = [
        {
            "x": x[i * ROWS : (i + 1) * ROWS],
            "W1": W1,
            "b1": b1,
            "W2": W2,
            "b2": b2,
        }
        for i in range(N_CORES)
    ]
    res = bass_utils.run_bass_kernel_spmd(
        nc, in_maps, core_ids=list(range(N_CORES)), trace=False
    )
    return np.concatenate([res.results[i]["out"] for i in range(N_CORES)], axis=0)
